# revision 38
# baseline (speedup 1.0000x reference)
"""Trainium2 Bass kernel for nn_AutoregressiveDecoder (8-core data parallel).

Strategy (v4):
  - Pure data parallel: B=16384 rows sharded 2048/core across 8 NeuronCores.
  - MLP compute runs feature-major (features on partitions, batch on the free
    dim) so weights act as the matmul stationary operand.
  - seq_embed @ w1[:512] is step-invariant -> computed once per 512-row
    macro-tile ("base", fp8 DoubleRow), stored as fp8 next to a per-step
    "extra features" zone in the same tile.  Each step's full L1 is then ONE
    DoubleRow matmul per 128-unit chunk: K-pair j=0 contracts the 15 extra
    rows against W1x (b1 folded into the one-hot rows), j=1 re-adds the base
    chunk through an identity stationary.
  - L2 runs fp8-e4m3 DoubleRow; h1 is written as fp8 by the gelu ACT pass
    directly (FD=1024 psum pairs, pres|fe).  L3 and h2 stay bf16 (DoubleRow
    is rejected for <128-partition psum dsts and plain fp8 gave no speedup).
  - The step loop is PAIR-SKEWED stage-major: macro-tiles are processed two
    at a time through [L1, L2, L3, transpose+plumbing+next-step-prep]
    stages.  The PE queue is strictly in-order, so this keeps one pair's PE
    stages executing while the other pair's DVE plumbing chains run.
  - Per-row scalar plumbing runs in a blocked batch-major layout
    [32 partitions, 16 blocks x 32 slots] bridged with 32x32 DVE
    StreamTransposes; state uses r-major (F,P,E,FL) quads so masked
    scatters and selects batch 3-4 slots per instruction with broadcast
    masks; logit|fe transposes share one [64, x] instruction.
  - DMA descriptor generation is minimized (~0.6us/descriptor of sequencer
    time): step inputs are host-packed into 2 tensors, outputs into 2
    tensors, seq loads are single 3D-AP descriptors, and issuance is spread
    across the sync/scalar/gpsimd queues.
  - Index-only preprocessing (ALL_PERMS lookup, one-hot, gathers) happens
    host-side in numpy; loss partial sums are reduced host-side.
"""

import numpy as np
import ml_dtypes

import concourse.bass as bass
import concourse.bacc as bacc
import concourse.tile as tile
from concourse import mybir
from concourse.bass_utils import run_bass_kernel_spmd

BF16 = mybir.dt.bfloat16
F32 = mybir.dt.float32
F8 = mybir.dt.float8e4
AF = mybir.ActivationFunctionType
ALU = mybir.AluOpType
DR = mybir.MatmulPerfMode.DoubleRow
NP_BF16 = ml_dtypes.bfloat16
NP_F8 = ml_dtypes.float8_e4m3

B, D, H = 16384, 512, 512
NCORES = 8
NB = 512            # macro-tile rows (matmul free dim)
ALL_PERMS = np.array(
    [[0, 1, 2], [0, 2, 1], [1, 0, 2], [1, 2, 0], [2, 0, 1], [2, 1, 0]], np.int32
)

# state-tile slot map (32 slots per 32-row block):
#   slots 0-11:  (F_r, P_r, E_r, FL_r) r-major quads
# st_bf / exT feature-major row order (what W1x contracts against):
#   rows 0-8: P(3), FL(3), ROH(3);  rows 9-14: F(3), E(3)
# pb scratch slots: 0-3 (a_f, a_p, a_e, one), 4-6 (pfc, sig, pec)


def r3(t, s):
    """view a [32, 16*s] tile as [32 p, 16 j, s slots]"""
    return t[:, :].rearrange("p (j s) -> p j s", s=s)


def quad(t, k):
    """r-major quad view: [32, 16 j, 3 r] AP over slots {k, k+4, k+8}."""
    return (
        r3(t, 32)[:, :, 0:12]
        .rearrange("p j (r q) -> p j r q", q=4)[:, :, :, k]
    )


def build_graph(BL, zbias=False):
    """Build the per-core Bass graph. BL = rows per core (multiple of NB).
    zbias: compile the variant for all-zero b2/b3 biases."""
    NM = BL // NB          # macro-tiles per core
    NBLK = NB // 32        # 32-row blocks per macro-tile (16)
    BLKT = BL // 32        # total blocks per core

    nc = bacc.Bacc("TRN2", target_bir_lowering=False, debug=False,
                   num_devices=NCORES)

    # ---- dram parameters -------------------------------------------------
    U8 = mybir.dt.uint8
    seq_d = nc.dram_tensor("seq", [D, BL], F8, kind="ExternalInput").ap()
    # packed per-(step,mt) inputs: [gt(3) | roh(3) | mask(1)] f32; the u8
    # predicate copies are derived on-device (saves 12 DMA descriptors)
    gr_d = nc.dram_tensor("gr", [96, BLKT * 7], F32, kind="ExternalInput").ap()

    pw1_d = nc.dram_tensor("pw1dr", [128, 2048], F8, kind="ExternalInput").ap()
    fw1_d = nc.dram_tensor("fw1dr", [128, 2048], F8, kind="ExternalInput").ap()
    pwe_d = nc.dram_tensor("pwe1", [128, 1024], F8, kind="ExternalInput").ap()
    fwe_d = nc.dram_tensor("fwe1", [128, 1024], F8, kind="ExternalInput").ap()
    pw2_d = nc.dram_tensor("pw2dr", [128, 1024], F8, kind="ExternalInput").ap()
    fw2_d = nc.dram_tensor("fw2dr", [128, 2048], F8, kind="ExternalInput").ap()
    pb2_d = nc.dram_tensor("pb2", [H // 2], F32, kind="ExternalInput").ap()
    fb2_d = nc.dram_tensor("fb2", [H], F32, kind="ExternalInput").ap()
    pw3_d = nc.dram_tensor("pw3b", [128, 64], BF16, kind="ExternalInput").ap()
    fw3_d = nc.dram_tensor("fw3b", [128, 128], BF16, kind="ExternalInput").ap()
    b3s_d = nc.dram_tensor("b3s", [1, 3], F32, kind="ExternalInput").ap()

    # packed outputs in the blocked on-chip layout (contiguous DMA; the
    # host un-blocks): dout = [df|dp|de], lpe = [lg|pf|pe]
    do_d = nc.dram_tensor("dout", [32, BLKT * 9], F32,
                          kind="ExternalOutput").ap()
    lp_d = nc.dram_tensor("lpe", [32, BLKT * 9], F32,
                          kind="ExternalOutput").ap()

    v = nc.vector
    sc = nc.scalar
    te = nc.tensor

    with tile.TileContext(nc) as tc:
        wpool = tc.alloc_tile_pool(name="w", bufs=1)
        pers = tc.alloc_tile_pool(name="pers", bufs=1)
        big1 = tc.alloc_tile_pool(name="big1", bufs=3)
        big2 = tc.alloc_tile_pool(name="big2", bufs=3)
        stp = tc.alloc_tile_pool(name="stp", bufs=2)
        smp = tc.alloc_tile_pool(name="smp", bufs=5)
        pp = tc.alloc_tile_pool(name="pspair", bufs=3, space="PSUM")
        ps3 = tc.alloc_tile_pool(name="psum3", bufs=2, space="PSUM")

        # ---- weights needed for phase 1 first (sync queue) --------------
        pw1_sb = wpool.tile([128, 2048], F8)
        fw1_sb = wpool.tile([128, 2048], F8)
        seqTs = {}
        nc.sync.dma_start(pw1_sb[:, :], pw1_d[:, :])
        # first macro-tile's seq rows race ahead of everything else so the
        # PE can start within ~2 descriptors of kernel start
        seqTs[0] = stp.tile([128, 4 * NB], F8, tag="seqT", name="seqT0")
        nc.sync.dma_start(
            seqTs[0][:, :].rearrange("p (j n) -> p j n", n=NB),
            seq_d[:, 0:NB].rearrange("(j p) n -> p j n", p=128))
        nc.sync.dma_start(fw1_sb[:, :], fw1_d[:, :])
        for mt in range(1, NM):
            seqTs[mt] = stp.tile([128, 4 * NB], F8, tag="seqT",
                                 name=f"seqT{mt}")
            nc.sync.dma_start(
                seqTs[mt][:, :].rearrange("p (j n) -> p j n", n=NB),
                seq_d[:, mt * NB:(mt + 1) * NB].rearrange(
                    "(j p) n -> p j n", p=128))
        # remaining weights on the scalar queue (issues in parallel)
        pwe_sb = wpool.tile([128, 1024], F8)
        fwe_sb = wpool.tile([128, 1024], F8)
        pw2_sb = wpool.tile([128, 1024], F8)
        fw2_sb = wpool.tile([128, 2048], F8)
        nc.scalar.dma_start(pwe_sb[:, :], pwe_d[:, :])
        nc.scalar.dma_start(fwe_sb[:, :], fwe_d[:, :])
        nc.scalar.dma_start(pw2_sb[:, :], pw2_d[:, :])
        nc.scalar.dma_start(fw2_sb[:, :], fw2_d[:, :])
        pw3_sb = wpool.tile([128, 64], BF16)
        fw3_sb = wpool.tile([128, 128], BF16)
        nc.scalar.dma_start(pw3_sb[:, :], pw3_d[:, :])
        nc.scalar.dma_start(fw3_sb[:, :], fw3_d[:, :])
        # preload the gelu table set at t~0 (copy/tanh ride in the same
        # set), so no ACT_TABLE_LOAD lands mid-kernel
        warm = wpool.tile([1, 8], F32)
        v.memset(warm[:, :], 0.0)
        sc.activation(warm[:, 0:4], warm[:, 4:8], AF.Gelu)
        # spin the PE during the DMA-bound head so the HAM clock gate is
        # already at 8/8 when phase 1 starts (~3.4us of sustained activity)
        wmm = wpool.tile([128, NB], BF16)
        v.memset(wmm[:, :], 0.0)
        wps = pp.tile([128, 2 * NB], F32, tag="ps_pair")
        for _ in range(10):
            te.matmul(wps[:, 0:NB], wmm[:, 0:128], wmm[:, :],
                      start=True, stop=True)
        if not zbias:
            pb2_sb = wpool.tile([128, 2], F32)
            fb2_sb = wpool.tile([128, 4], F32)
            nc.scalar.dma_start(pb2_sb[:, :],
                                pb2_d.rearrange("(m p) -> p m", p=128))
            nc.scalar.dma_start(fb2_sb[:, :],
                                fb2_d.rearrange("(m p) -> p m", p=128))
            b3s_sb = wpool.tile([1, 3], F32)
            nc.scalar.dma_start(b3s_sb[:, :], b3s_d[:, :])
            b3bc = wpool.tile([32, 3], F32)
            nc.gpsimd.partition_broadcast(b3bc[:, :], b3s_sb[:, :])

        def w1s(t, a, c):          # phase-1 w1 slice [128, 2, 128]
            off = (a * 4 + c) * 256
            return t[:, off:off + 256].rearrange("p (j m) -> p j m", m=128)

        def wes(t, m):             # L1 extra+eye slice [128, 2, 128]
            return t[:, m * 256:(m + 1) * 256].rearrange(
                "p (j m) -> p j m", m=128)

        def pw2s(a, c):
            off = (a * 2 + c) * 256
            return pw2_sb[:, off:off + 256].rearrange("p (j m) -> p j m", m=128)

        def fw2s(a, c):
            off = (a * 4 + c) * 256
            return fw2_sb[:, off:off + 256].rearrange("p (j m) -> p j m", m=128)

        views = {0: {}, 1: {}, 2: {}}

        def emit_inputs_and_bridge(s, mt):
            stv = r3(sts[mt], 32)
            sbv = r3(stbf[mt], 32)
            gr_sb = smp.tile([32, NBLK * 7], F32, tag="gr", name=f"gr{s}_{mt}")
            nc.gpsimd.dma_start(
                gr_sb[:, :],
                gr_d[s * 32:(s + 1) * 32,
                     mt * NBLK * 7:(mt + 1) * NBLK * 7])
            gt3 = gr_sb[:, 0:NBLK * 3].rearrange("p (j s) -> p j s", s=3)
            roh3 = gr_sb[:, NBLK * 3:NBLK * 6].rearrange(
                "p (j s) -> p j s", s=3)
            mr_sb = smp.tile([32, NBLK * 4], U8, tag="mr", name=f"mr{s}_{mt}")
            v.tensor_copy(mr_sb[:, :], gr_sb[:, NBLK * 3:NBLK * 7])
            roi3 = mr_sb[:, 0:NBLK * 3].rearrange("p (j s) -> p j s", s=3)
            mi1 = mr_sb[:, NBLK * 3:NBLK * 4].rearrange(
                "p (j s) -> p j s", s=1)

            if s > 0:
                v.tensor_copy(sbv[:, :, 0:3], quad(sts[mt], 1))   # P
                v.tensor_copy(sbv[:, :, 3:6], quad(sts[mt], 3))   # FL
                v.tensor_copy(sbv[:, :, 9:12], quad(sts[mt], 0))  # F
                v.tensor_copy(sbv[:, :, 12:15], quad(sts[mt], 2)) # E
            v.tensor_copy(sbv[:, :, 6:9], roh3[:, :, :])          # ROH
            exT = smp.tile([32, NBLK * 32], BF16, tag="exT",
                           name=f"exT{s}_{mt}")
            v.transpose(exT[0:32, :], stbf[mt][:, :])
            # refresh the shared extra-feature zone (bf16 -> fp8)
            v.tensor_copy(bse[mt][0:15, 0:NB], exT[0:15, :])
            views[s][mt] = (gt3, roh3, mi1, roi3)


        # ---- phase 1: all macro-tile bases (fp8 DoubleRow) --------------
        # bse layout [128, 16*NB] fp8: cols 0:NB = shared extra-feature zone
        # (rows 0-14 live, rest zeroed), pres chunk m at (1+m)*NB, fe chunk
        # m at (5+m)*NB.  The L1 DoubleRow moving AP [p, 2, NB] has j-stride
        # = the chunk's column offset, so j=0 is always the extra zone.
        bse, sts, stbf, lgos = {}, {}, {}, {}
        for mt in range(NM):
            seqr = seqTs[mt][:, :].rearrange("p (j n) -> p j n", n=NB)
            bse[mt] = pers.tile([128, 16 * NB], F8, tag=f"bse{mt}",
                                name=f"bse{mt}")
            v.memset(bse[mt][:, 0:NB], 0.0)
            for ni, coff in enumerate((NB, 5 * NB)):
                wsb = pw1_sb if ni == 0 else fw1_sb
                for pair in range(2):
                    psa = pp.tile([128, 2 * NB], F32, tag="ps_pair")
                    for ci in range(2):
                        c = 2 * pair + ci
                        for a in range(2):
                            te.matmul(psa[:, ci * NB:(ci + 1) * NB],
                                      w1s(wsb, a, c),
                                      seqr[:, 2 * a:2 * a + 2, :],
                                      start=(a == 0), stop=(a == 1),
                                      perf_mode=DR)
                    # split the psum->sbuf casts between ACT and DVE so
                    # neither engine gates phase 1
                    dst = bse[mt][:, coff + 2 * pair * NB:
                                  coff + (2 * pair + 2) * NB]
                    if ni == 0:
                        sc.copy(dst, psa[:, :])
                    else:
                        v.tensor_copy(dst, psa[:, :])
            sts[mt] = pers.tile([32, NBLK * 32], F32, tag=f"st{mt}",
                                name=f"st{mt}")
            v.memset(sts[mt][:, :], 0.0)
            stbf[mt] = pers.tile([32, NBLK * 32], BF16, tag=f"stbf{mt}",
                                 name=f"stbf{mt}")
            v.memset(stbf[mt][:, :], 0.0)
            lgos[mt] = pers.tile([32, NBLK * 9], F32, tag=f"lgo{mt}",
                                 name=f"lgo{mt}")
            emit_inputs_and_bridge(0, mt)

        # ---- phase 2: autoregressive steps ------------------------------
        pairs = [[g * 2, g * 2 + 1] for g in range(NM // 2)] or [[0]]
        for s in range(3):
            for mts in pairs:
                h1s, h2s, p3s = {}, {}, {}
                # -- stage 1: L1 (one DoubleRow matmul per chunk and net:
                # j=0 extras @ W1x, j=1 base re-add via identity) + gelu(fp8)
                for mt in mts:
                    h1s[mt] = big1.tile([128, 8 * NB], F8, tag="h1",
                                        name=f"h1_{mt}")
                for m in range(4):
                    for mt in mts:
                        xps = pp.tile([128, 2 * NB], F32, tag="ps_pair")
                        wp = (m + 1) * NB
                        mvp = bse[mt][:, 0:2 * wp].rearrange(
                            "p (j n) -> p j n", n=wp)[:, :, 0:NB]
                        te.matmul(xps[:, 0:NB], wes(pwe_sb, m), mvp,
                                  start=True, stop=True, perf_mode=DR)
                        wf = (5 + m) * NB
                        mvf = bse[mt][:, 0:2 * wf].rearrange(
                            "p (j n) -> p j n", n=wf)[:, :, 0:NB]
                        te.matmul(xps[:, NB:2 * NB], wes(fwe_sb, m), mvf,
                                  start=True, stop=True, perf_mode=DR)
                        h1v = h1s[mt][:, :].rearrange(
                            "p (net m n) -> p m net n", net=2, n=NB)
                        sc.activation(h1v[:, m:m + 1].rearrange(
                                          "p m net n -> p (m net) n"),
                                      xps[:, :].rearrange(
                                          "p (net n) -> p net n", n=NB),
                                      AF.Gelu)

                # -- stage 2: L2 (fp8 DoubleRow) -> gelu(bf16) (+ b2 bias)
                for mt in mts:
                    h1r = h1s[mt][:, :].rearrange("p (j n) -> p j n", n=NB)
                    h2s[mt] = big2.tile([128, 6 * NB], BF16, tag="h2",
                                        name=f"h2_{mt}")
                    h2 = h2s[mt]
                    ps2 = pp.tile([128, 2 * NB], F32, tag="ps_pair")
                    for ci in range(2):
                        for a in range(2):
                            te.matmul(ps2[:, ci * NB:(ci + 1) * NB],
                                      pw2s(a, ci),
                                      h1r[:, 2 * a:2 * a + 2, :],
                                      start=(a == 0), stop=(a == 1),
                                      perf_mode=DR)
                    if zbias:
                        sc.activation(h2[:, 0:2 * NB], ps2[:, :], AF.Gelu)
                    else:
                        for ci in range(2):
                            sc.activation(h2[:, ci * NB:(ci + 1) * NB],
                                          ps2[:, ci * NB:(ci + 1) * NB],
                                          AF.Gelu, bias=pb2_sb[:, ci:ci + 1])
                    for q in range(2):
                        ps2 = pp.tile([128, 2 * NB], F32, tag="ps_pair")
                        for ci in range(2):
                            c = 2 * q + ci
                            for a in range(2):
                                te.matmul(ps2[:, ci * NB:(ci + 1) * NB],
                                          fw2s(a, c),
                                          h1r[:, 4 + 2 * a:6 + 2 * a, :],
                                          start=(a == 0), stop=(a == 1),
                                          perf_mode=DR)
                        if zbias:
                            sc.activation(
                                h2[:, (2 + 2 * q) * NB:(4 + 2 * q) * NB],
                                ps2[:, :], AF.Gelu)
                        else:
                            for ci in range(2):
                                c = 2 * q + ci
                                sc.activation(
                                    h2[:, (2 + c) * NB:(3 + c) * NB],
                                    ps2[:, ci * NB:(ci + 1) * NB],
                                    AF.Gelu, bias=fb2_sb[:, c:c + 1])

                # -- stage 3: L3 (bf16, padded to M=32; pres in col-group
                # 0, fe in col-group 1 of the same psum tile)
                for mt in mts:
                    h2r = h2s[mt][:, :].rearrange("p (j n) -> p j n", n=NB)
                    p3s[mt] = ps3.tile([64, NB], F32, tag="ps3",
                                       name=f"p3_{mt}")
                    p3 = p3s[mt]
                    # interleave pres/fe so consecutive MMs hit different
                    # col-groups and overlap in the PE array
                    for kk in range(4):
                        if kk < 2:
                            te.matmul(p3[0:32, :],
                                      pw3_sb[:, kk * 32:(kk + 1) * 32],
                                      h2r[:, kk], start=(kk == 0),
                                      stop=(kk == 1))
                        te.matmul(p3[32:64, :],
                                  fw3_sb[:, kk * 32:(kk + 1) * 32],
                                  h2r[:, 2 + kk], start=(kk == 0),
                                  stop=(kk == 3), tile_position=(0, 32))

                # -- stage 4: transpose + per-row plumbing + state scatter,
                # fused with step s+1's input prep per macro-tile
                for mt in mts:
                    gt3, roh3, mi1, roi3 = views[s][mt]
                    stv = r3(sts[mt], 32)
                    # one [64, x] StreamTranspose covers logit and fe rows
                    lfT = smp.tile([64, NBLK * 32], F32, tag="lfT",
                                   name=f"lfT_{mt}")
                    v.transpose(lfT[:, :], p3s[mt][:, :])
                    lg3 = lfT[0:32, :].rearrange("p (j s) -> p j s", s=32)
                    fe3 = lfT[32:64, :].rearrange("p (j s) -> p j s", s=32)
                    logit = lg3[:, :, 0:1]
                    pf = fe3[:, :, 0:1]
                    pe = fe3[:, :, 1:2]
                    if not zbias:
                        v.tensor_scalar_add(logit, logit, b3bc[:, 0:1])
                        v.tensor_scalar_add(pf, pf, b3bc[:, 1:2])
                        v.tensor_scalar_add(pe, pe, b3bc[:, 2:3])

                    pb = smp.tile([32, NBLK * 8], F32, tag="pb")
                    pb3d = r3(pb, 8)
                    a_qd = pb3d[:, :, 0:4]
                    pfc, sig, pec = (pb3d[:, :, 4:5], pb3d[:, :, 5:6],
                                     pb3d[:, :, 6:7])
                    v.memset(pb3d[:, :, 3:4], 1.0)   # decoded flag

                    # raw outputs for host loss: lgos layout (j, kind, s)
                    lp4 = lgos[mt][:, :].rearrange("p (j k s) -> p j k s",
                                                   k=3, s=3)
                    v.tensor_copy(lp4[:, :, 0, s], lg3[:, :, 0])
                    v.tensor_copy(lp4[:, :, 1:3, s], fe3[:, :, 0:2])

                    # sigmoid(l) = 0.5*tanh(0.5*l) + 0.5 (gelu table set)
                    sc.activation(sig, logit, AF.Tanh, scale=0.5)
                    v.tensor_scalar(sig, sig, 0.5, 0.5, ALU.mult, ALU.add)
                    v.tensor_scalar(pfc, pf, -10.0, 10.0, ALU.max, ALU.min)
                    v.tensor_scalar(pec, pe, -100.0, 100.0, ALU.max, ALU.min)
                    # a = mask ? clipped-prediction : ground-truth (f, p, e)
                    mib = mi1[:, :, 0:1].broadcast_to((32, NBLK, 3))
                    v.select(pb3d[:, :, 0:3], mib, pb3d[:, :, 4:7],
                             gt3[:, :, 0:3])

                    # state scatter: quad r = (acts, 1) where roh_r else keep
                    for r in range(3):
                        rp4 = roi3[:, :, r:r + 1].broadcast_to(
                            (32, NBLK, 4))
                        v.copy_predicated(stv[:, :, 4 * r:4 * r + 4],
                                          rp4, a_qd)

                    if s < 2:
                        emit_inputs_and_bridge(s + 1, mt)
                    else:
                        cols = slice(mt * NBLK * 9, (mt + 1) * NBLK * 9)
                        # repack stride-4 quads into one contiguous staging
                        # tile (DMA needs a contiguous inner dim)
                        dout = smp.tile([32, NBLK * 9], F32, tag="dout")
                        do4 = dout[:, :].rearrange("p (j k r) -> p j k r",
                                                   k=3, r=3)
                        for k in range(3):
                            v.tensor_copy(do4[:, :, k], quad(sts[mt], k))
                        nc.sync.dma_start(do_d[:, cols], dout[:, :])
                        nc.gpsimd.dma_start(lp_d[:, cols], lgos[mt][:, :])

        for p in (ps3, pp, smp, stp, big2, big1, pers, wpool):
            p.release()

    nc.compile()
    return nc


# ---------------------------------------------------------------------------
def _dr_pack(w, mc):
    """Pack [K, M] weights into the DoubleRow stationary layout
    [128, (K//256) * (M//mc) * 2 * mc] with index (a, c, jj, m)."""
    K, M = w.shape
    a, c = K // 256, M // mc
    v = w.reshape(a, 2, 128, c, mc)          # [a, jj, p, c, m]
    v = v.transpose(2, 0, 3, 1, 4)           # [p, a, c, jj, m]
    return np.ascontiguousarray(v.reshape(128, a * c * 2 * mc))


def _we_pack(w1x):
    """Pack the L1 extra weights + identity into the DoubleRow stationary
    layout [128, 4 * 2 * 128] with index (m, j, mc):
    j=0 -> W1x rows (padded to 128), j=1 -> I."""
    nrows = w1x.shape[0]
    out = np.zeros((128, 4, 2, 128), np.float32)
    eye = np.eye(128, dtype=np.float32)
    for m in range(4):
        out[0:nrows, m, 0, :] = w1x[:, m * 128:(m + 1) * 128]
        out[:, m, 1, :] = eye
    return out.reshape(128, 1024)


def prep_inputs(seq_embed, freq, pres, enrich,
                pw1, pb1, pw2, pb2, pw3, pb3,
                fw1, fb1, fw2, fb2, fw3, fb3,
                perm_idx, round_mask, BL):
    """Host-side (numpy) sharding + index preprocessing."""
    f32 = np.float32
    seq = np.asarray(seq_embed, f32)
    perms = ALL_PERMS[np.asarray(perm_idx)]                    # [B,3]
    gtf = np.take_along_axis(np.asarray(freq, f32), perms, 1)   # [B,3] (col=s)
    gtp = np.take_along_axis(np.asarray(pres, f32), perms, 1)
    gte = np.take_along_axis(np.asarray(enrich, f32), perms, 1)
    m = np.take_along_axis(np.asarray(round_mask), perms, 1).astype(f32)
    roh = (perms[:, :, None] == np.arange(3)[None, None, :]).astype(f32)  # [B,3s,3r]

    bf = lambda a: np.ascontiguousarray(np.asarray(a, f32).astype(NP_BF16))
    f8 = lambda a: np.ascontiguousarray(np.asarray(a, f32).astype(NP_F8))
    # W1x rows in exT order (P, FL, ROH, F, E); b1 folded into the ROH rows
    pw1x = np.asarray(pw1, f32)[512:521][[0, 2, 4, 1, 3, 5, 6, 7, 8]].copy()
    pw1x[6:9] += np.asarray(pb1, f32)[None, :]
    fw1x = np.asarray(fw1, f32)[512:527][
        [1, 5, 9, 3, 7, 11, 12, 13, 14, 0, 4, 8, 2, 6, 10]].copy()
    fw1x[6:9] += np.asarray(fb1, f32)[None, :]
    pw3p = np.zeros((256, 32), f32); pw3p[:, 0] = np.asarray(pw3, f32)[:, 0]
    fw3p = np.zeros((512, 32), f32); fw3p[:, 0:2] = np.asarray(fw3, f32)
    b3s = np.array([[np.asarray(pb3, f32)[0],
                     np.asarray(fb3, f32)[0], np.asarray(fb3, f32)[1]]], f32)

    shared = {
        "pw1dr": f8(_dr_pack(np.asarray(pw1, f32)[:512], 128)),
        "fw1dr": f8(_dr_pack(np.asarray(fw1, f32)[:512], 128)),
        "pwe1": f8(_we_pack(pw1x)),
        "fwe1": f8(_we_pack(fw1x)),
        "pw2dr": f8(_dr_pack(np.asarray(pw2, f32), 128)),
        "fw2dr": f8(_dr_pack(np.asarray(fw2, f32), 128)),
        "pb2": np.ascontiguousarray(np.asarray(pb2, f32)),
        "fb2": np.ascontiguousarray(np.asarray(fb2, f32)),
        "pw3b": bf(_dr_pack(pw3p, 32)),
        "fw3b": bf(_dr_pack(fw3p, 32)),
        "b3s": b3s,
    }

    in_maps = []
    ncores = seq.shape[0] // BL
    BLKT = BL // 32
    NBLK = 16
    nmt = BLKT // NBLK
    for c in range(ncores):
        rs = slice(c * BL, (c + 1) * BL)
        # blocked layouts: index [s*32+p, Jg*w + q], b_local = 32*Jg + p
        gt3 = np.stack([gtf[rs], gtp[rs], gte[rs]], -1)          # [BL,3s,3]
        gt3 = gt3.reshape(BLKT, 32, 3, 3).transpose(2, 1, 0, 3)  # [3s,32,J,3]
        rohc = roh[rs].reshape(BLKT, 32, 3, 3).transpose(2, 1, 0, 3)
        mc = m[rs].reshape(BLKT, 32, 3).transpose(2, 1, 0)       # [3s,32,J]
        # pack [gt | roh | mask] f32 per macro-tile block
        gr = np.zeros((3, 32, BLKT * 7), f32)
        for mt in range(nmt):
            js = slice(mt * NBLK, (mt + 1) * NBLK)
            o = mt * NBLK * 7
            gr[:, :, o:o + NBLK * 3] = gt3[:, :, js, :].reshape(3, 32, -1)
            gr[:, :, o + NBLK * 3:o + NBLK * 6] = rohc[:, :, js, :].reshape(
                3, 32, -1)
            gr[:, :, o + NBLK * 6:o + NBLK * 7] = mc[:, :, js]
        in_maps.append(dict(
            seq=np.ascontiguousarray(seq[rs].astype(NP_F8).T),
            gr=np.ascontiguousarray(gr.reshape(96, BLKT * 7)),
            **shared))
    aux = dict(gtf=gtf, gtp=gtp, gte=gte, m=m)
    return in_maps, aux


def assemble(results, aux):
    """Gather per-core outputs; finish the (tiny) loss reductions host-side."""
    f32 = np.float32
    def unblock(key):
        # [32, BLKT*9] blocked (p, j, k, r) -> [B, 3 k, 3 r]
        arrs = []
        for r in results:
            a = np.asarray(r[key], f32).reshape(32, -1, 3, 3)
            arrs.append(a.transpose(1, 0, 2, 3).reshape(-1, 3, 3))
        return np.concatenate(arrs, 0)
    do = unblock("dout")
    lpe = unblock("lpe")
    df, dp, de = do[:, 0, :], do[:, 1, :], do[:, 2, :]
    lg, pf, pe = lpe[:, 0, :], lpe[:, 1, :], lpe[:, 2, :]
    m, gtf, gtp, gte = aux["m"], aux["gtf"], aux["gtp"], aux["gte"]
    lf = np.sum(np.square(pf - gtf) * m, dtype=np.float64)
    le = np.sum(np.square(pe - gte) * m, dtype=np.float64)
    bce = (np.maximum(lg, 0.0) - lg * gtp
           + np.log1p(np.exp(-np.abs(lg), dtype=np.float64)))
    lp = np.sum(bce * m, dtype=np.float64)
    nm = np.sum(m, dtype=np.float64) + 1e-8
    head = np.array([lf / nm, lp / nm, le / nm], f32)
    return np.concatenate([head, df.ravel(), dp.ravel(), de.ravel()])


_CACHE = {}


def _get_graph(BL, zbias):
    key = (BL, zbias)
    if key not in _CACHE:
        _CACHE[key] = build_graph(BL, zbias)
    return _CACHE[key]


def _install_profile_hook():
    """Provide antenv.axon_hooks (missing in this image) so trace=True works."""
    import sys, types
    try:
        import antenv.axon_hooks  # noqa: F401
        return
    except ImportError:
        pass
    from trn_agent_boot.trn_boot import _ntff_profile_via_ctypes
    hook = _ntff_profile_via_ctypes('/opt/axon/libaxon_pjrt.so')
    mod = types.ModuleType('antenv.axon_hooks')
    mod._hook = hook
    mod.get_axon_ntff_profile_hook = lambda: mod._hook
    mod.set_axon_ntff_profile_hook = lambda h: setattr(mod, '_hook', h)
    sys.modules['antenv.axon_hooks'] = mod


def run(inputs, trace=False):
    if trace:
        _install_profile_hook()
    BL = inputs["seq_embed"].shape[0] // NCORES
    zbias = not any(np.any(np.asarray(inputs[k]))
                    for k in ("pb2", "fb2", "pb3", "fb3"))
    nc = _get_graph(BL, zbias)
    in_maps, aux = prep_inputs(**inputs, BL=BL)
    res = run_bass_kernel_spmd(nc, in_maps, core_ids=list(range(NCORES)),
                               trace=trace)
    out = assemble(res.results, aux)
    return out, res


def kernel(**inputs):
    inputs = {k: np.asarray(v) for k, v in inputs.items()}
    out, _ = run(inputs)
    return out
